# revision 1
# baseline (speedup 1.0000x reference)
"""Trainium2 Bass kernel for nn_Conv4Pim_group_arr_v3 (PIM-style grouped quantized conv).

Computation (see reference):
  - x [16,256,56,56] f32, weight [256,256,3,3], per-group (G=4, 64 ic each) LSQ
    quantization: weights to integer levels {0..3} (pos/neg split), partial-sum conv
    outputs rounded to int levels in [-128,127] and rescaled, accumulated over groups.

Strategy: data-parallel over batch (2 images per core, 8 cores, no collectives).
Per core, per (img, group, och-tile-of-512, sptile-of-8-rows):
  - 5 fp16 matmuls (K=128: two conv taps x 64 ic stacked; x stored as [A | A+1] and
    [A | A+58] shifted copies so taps pair up) accumulate the 3x3 conv into one
    PSUM tile [128 och, 464=8*58 padded-row columns].
  - ACT: Copy(psum * (w_scale/ps_scale)) with int8 output = round-half-even +
    saturate to [-128,127] in one op (verified on HW) == the LSQ psum quantizer.
  - DVE scalar_tensor_tensor: acc_fp16 += q_int8 * (+-ps_scale).
Output fp16 -> host f32.
"""

import numpy as np

import concourse.mybir as mybir
import concourse.tile as tile
from concourse import bacc
from concourse.bass_utils import run_bass_kernel_spmd

F32 = mybir.dt.float32
F16 = mybir.dt.float16
I8 = mybir.dt.int8

B, IC, H, W = 16, 256, 56, 56
OC = 256
G = 4
CG = 64  # ic per group
K = 3
QP_W = 3  # 2**2 - 1
N_CORES = 8
BPC = B // N_CORES  # images per core

PW = W + 2  # 58 padded width
PH = H + 2
FLAT = PW * PH  # 3364
FLATP = FLAT + 4  # padded to 3368 for tap-read overhang
SP = 7  # spatial tiles of 8 output rows
ROWS = 8
NCOL = ROWS * PW  # 464 columns per psum tile
OC4 = 4  # och tiles of 128 over 512 (pos|neg x 256)

_nc_cache = {}


def _build_nc():
    nc = bacc.Bacc(
        "TRN2",
        target_bir_lowering=False,
        debug=False,
        enable_asserts=True,
        num_devices=N_CORES,
    )

    xt1_d = nc.dram_tensor("xt1", [BPC, G, 128, FLATP], F16, kind="ExternalInput").ap()
    xt2_d = nc.dram_tensor("xt2", [BPC, G, 128, FLATP], F16, kind="ExternalInput").ap()
    wts_d = nc.dram_tensor("wts", [128, G * OC4 * 5 * 128], F16, kind="ExternalInput").ap()
    scl_d = nc.dram_tensor("scl", [128, 2 * G * OC4], F32, kind="ExternalInput").ap()
    # padded output: [img, oct, sp, och, 464 cols]; host strips the pad columns
    out_d = nc.dram_tensor("out", [BPC, 2, SP, 128, NCOL], F16, kind="ExternalOutput").ap()

    with tile.TileContext(nc) as tc:
        with (
            tc.tile_pool(name="xp", bufs=1) as xp,
            tc.tile_pool(name="wp", bufs=1) as wp,
            tc.tile_pool(name="accp", bufs=2) as accp,
            tc.tile_pool(name="qp", bufs=8) as qp,
            tc.tile_pool(name="psum", bufs=8, space="PSUM") as pp,
        ):
            wts = wp.tile([128, G * OC4 * 5 * 128], F16, tag="wts")
            scl = wp.tile([128, 2 * G * OC4], F32, tag="scl")
            # Startup-critical DMA schedule over two queues (sync = HWDGE, gpsimd =
            # SWDGE). The first (img0, g0) block runs sp-outer (see compute loop), so
            # only the first ~1100 cols of T1/T2[0,0] + the (g0,oc4=0) weight slice
            # gate the first matmul. Chunks ordered to stay ahead of consumption.
            W1 = 5 * 128  # one (g, oc4) weight slice
            WG = OC4 * W1  # one group
            C1, C2 = 1100, 2500  # x-tile column chunk boundaries

            xt = {}
            t1_first = xp.tile([128, FLATP], F16, tag="t1_0_0")
            t2_first = xp.tile([128, FLATP], F16, tag="t2_0_0")
            xt[0, 0] = (t1_first, t2_first)

            nc.sync.dma_start(scl[:], scl_d[:])
            nc.sync.dma_start(wts[:, :W1], wts_d[:, :W1])
            nc.sync.dma_start(t1_first[:, :C1], xt1_d[0, 0, :, :C1])
            for i in range(1, OC4):
                nc.sync.dma_start(wts[:, i * W1 : (i + 1) * W1], wts_d[:, i * W1 : (i + 1) * W1])
            nc.sync.dma_start(t1_first[:, C1:C2], xt1_d[0, 0, :, C1:C2])
            nc.sync.dma_start(t1_first[:, C2:], xt1_d[0, 0, :, C2:])

            nc.gpsimd.dma_start(t2_first[:, :C1], xt2_d[0, 0, :, :C1])
            nc.gpsimd.dma_start(t2_first[:, C1:C2], xt2_d[0, 0, :, C1:C2])
            nc.gpsimd.dma_start(t2_first[:, C2:], xt2_d[0, 0, :, C2:])
            nc.gpsimd.dma_start(wts[:, WG : 2 * WG], wts_d[:, WG : 2 * WG])

            for img in range(BPC):
                for g in range(G):
                    if (img, g) in xt:
                        continue
                    t1 = xp.tile([128, FLATP], F16, tag=f"t1_{img}_{g}")
                    t2 = xp.tile([128, FLATP], F16, tag=f"t2_{img}_{g}")
                    nc.sync.dma_start(t1[:], xt1_d[img, g])
                    nc.gpsimd.dma_start(t2[:], xt2_d[img, g])
                    xt[img, g] = (t1, t2)
                    if (img, g) == (0, 1):
                        # remaining weights after the (0,1) x tiles
                        nc.gpsimd.dma_start(wts[:, 2 * WG :], wts_d[:, 2 * WG :])

            def wslice(g, oc4, s):
                i = ((g * OC4) + oc4) * 5 + s
                return wts[:, i * 128 : (i + 1) * 128]

            for img in range(BPC):
                acc = {}
                for oct in range(2):
                    for sp in range(SP):
                        a_t = accp.tile([128, NCOL], F16, tag=f"acc{oct}_{sp}")
                        acc[oct, sp] = a_t

                for g in range(G):
                    t1, t2 = xt[img, g]
                    if img == 0 and g == 0:
                        # sp-outer so the first matmuls need only the first x chunk
                        combos = [(oc4, sp) for sp in range(SP) for oc4 in range(OC4)]
                    else:
                        combos = [(oc4, sp) for oc4 in range(OC4) for sp in range(SP)]
                    for oc4, sp in combos:
                        iscl = g * OC4 + oc4
                        ratio_ap = scl[:, iscl : iscl + 1]
                        c_ap = scl[:, G * OC4 + iscl : G * OC4 + iscl + 1]
                        if True:
                            r0 = sp * ROWS
                            ps = pp.tile([128, NCOL], F32, tag="ps")
                            for s in range(3):
                                nc.tensor.matmul(
                                    ps[:],
                                    wslice(g, oc4, s),
                                    t1[:, (r0 + s) * PW : (r0 + s) * PW + NCOL],
                                    start=(s == 0),
                                    stop=False,
                                )
                            nc.tensor.matmul(
                                ps[:],
                                wslice(g, oc4, 3),
                                t2[:, r0 * PW + 2 : r0 * PW + 2 + NCOL],
                                start=False,
                                stop=False,
                            )
                            nc.tensor.matmul(
                                ps[:],
                                wslice(g, oc4, 4),
                                t1[:, (r0 + 2) * PW + 2 : (r0 + 2) * PW + 2 + NCOL],
                                start=False,
                                stop=True,
                            )
                            q8 = qp.tile([128, NCOL], I8, tag="q8")
                            nc.scalar.activation(
                                q8[:],
                                ps[:],
                                mybir.ActivationFunctionType.Copy,
                                bias=0.0,
                                scale=ratio_ap,
                            )
                            a = acc[oc4 % 2, sp]
                            if g == 0 and oc4 < 2:
                                nc.vector.tensor_scalar(
                                    a[:], q8[:], c_ap, None, mybir.AluOpType.mult
                                )
                            else:
                                nc.vector.scalar_tensor_tensor(
                                    a[:],
                                    q8[:],
                                    c_ap,
                                    a[:],
                                    mybir.AluOpType.mult,
                                    mybir.AluOpType.add,
                                )

                for oct in range(2):
                    for sp in range(SP):
                        a = acc[oct, sp]
                        eng = nc.sync if (oct * SP + sp) % 2 == 0 else nc.gpsimd
                        eng.dma_start(out_d[img, oct, sp], a[:])

    nc.compile()
    return nc


def _prepare(x, weight, w_scale, ps_scale_p, ps_scale_n):
    x = np.asarray(x, np.float32)
    weight = np.asarray(weight, np.float32)
    w_scale = np.asarray(w_scale, np.float32)
    ps_scale_p = np.asarray(ps_scale_p, np.float32)
    ps_scale_n = np.asarray(ps_scale_n, np.float32)

    # --- weight levels (exact f32 math matching the reference LSQ) ---
    wg = weight.reshape(OC, G, CG, K, K).transpose(1, 0, 2, 3, 4)  # [G,O,cg,k,k]
    s_w = w_scale.reshape(G, 1, 1, 1, 1)
    lvl_p = np.round(np.clip(np.maximum(wg, 0) / s_w, 0.0, float(QP_W))).astype(np.float32)
    lvl_n = np.round(np.clip(np.maximum(-wg, 0) / s_w, 0.0, float(QP_W))).astype(np.float32)
    LV = np.concatenate([lvl_p, lvl_n], axis=1)  # [G, 512, cg, 3, 3]

    # lhsT tiles [K=128, M=128] per (g, oc4, slot)
    wts = np.zeros((G, OC4, 5, 128, 128), np.float16)
    for g in range(G):
        for oc4 in range(OC4):
            t = LV[g, oc4 * 128 : (oc4 + 1) * 128]  # [128 och, cg, 3, 3]
            for s in range(3):  # taps (s,0)+(s,1)
                wts[g, oc4, s, :CG] = t[:, :, s, 0].T
                wts[g, oc4, s, CG:] = t[:, :, s, 1].T
            wts[g, oc4, 3, :CG] = t[:, :, 0, 2].T  # taps (0,2)+(1,2) via T2
            wts[g, oc4, 3, CG:] = t[:, :, 1, 2].T
            wts[g, oc4, 4, :CG] = t[:, :, 2, 2].T  # tap (2,2), upper half zero
    # -> [128 K, G*OC4*5*128]
    wts_flat = np.ascontiguousarray(wts.transpose(3, 0, 1, 2, 4).reshape(128, G * OC4 * 5 * 128))

    # --- scales: ratio = s_w/s_ps ; c = +-s_ps ---
    scl = np.zeros((128, 2 * G * OC4), np.float32)
    for g in range(G):
        for oc4 in range(OC4):
            s_ps = ps_scale_p[g] if oc4 < 2 else ps_scale_n[g]
            sign = 1.0 if oc4 < 2 else -1.0
            scl[:, g * OC4 + oc4] = np.float32(w_scale[g]) / np.float32(s_ps)
            scl[:, G * OC4 + g * OC4 + oc4] = np.float32(sign) * np.float32(s_ps)

    # --- padded, shifted x in fp16 ---
    xp = np.zeros((B, IC, PH, PW), np.float16)
    xp[:, :, 1 : H + 1, 1 : W + 1] = x.astype(np.float16)
    Af = np.zeros((B, G, CG, FLATP), np.float16)
    Af[..., :FLAT] = xp.reshape(B, G, CG, FLAT)
    T1 = np.zeros((B, G, 128, FLATP), np.float16)
    T1[:, :, :CG] = Af
    T1[:, :, CG:, : FLATP - 1] = Af[..., 1:]
    T2 = np.zeros((B, G, 128, FLATP), np.float16)
    T2[:, :, :CG] = Af
    T2[:, :, CG:, : FLATP - PW] = Af[..., PW:]

    return T1, T2, wts_flat, scl


def kernel(x, weight, w_scale, ps_scale_p, ps_scale_n, _trace=False, _tmpdir=None):
    T1, T2, wts_flat, scl = _prepare(x, weight, w_scale, ps_scale_p, ps_scale_n)

    if "nc" not in _nc_cache:
        _nc_cache["nc"] = _build_nc()
    nc = _nc_cache["nc"]

    in_maps = []
    for c in range(N_CORES):
        sl = slice(c * BPC, (c + 1) * BPC)
        in_maps.append(
            {
                "xt1": np.ascontiguousarray(T1[sl]),
                "xt2": np.ascontiguousarray(T2[sl]),
                "wts": wts_flat,
                "scl": scl,
            }
        )

    kwargs = {}
    if _trace:
        kwargs.update(trace=True, tmpdir=_tmpdir, trace_cores=[0])
    res = run_bass_kernel_spmd(nc, in_maps, core_ids=list(range(N_CORES)), **kwargs)

    out = np.concatenate([r["out"] for r in res.results], axis=0)  # [16, 2, 7, 128, 464] fp16
    v = out.reshape(B, 2, SP, 128, ROWS, PW)[..., :W]  # strip pad cols
    final = np.ascontiguousarray(v.transpose(0, 1, 3, 2, 4, 5)).reshape(B, OC, H, W).astype(np.float32)
    if _trace:
        kernel._last_results = res
    return final



# revision 3
# speedup vs baseline: 1.0149x; 1.0149x over previous
"""Trainium2 Bass kernel for nn_Conv4Pim_group_arr_v3 (PIM-style grouped quantized conv).

Computation (see reference):
  - x [16,256,56,56] f32, weight [256,256,3,3], per-group (G=4, 64 ic each) LSQ
    quantization: weights to integer levels {0..3} (pos/neg split), partial-sum conv
    outputs rounded to int levels in [-128,127] and rescaled, accumulated over groups.

Strategy: data-parallel over batch (2 images per core, 8 cores, no collectives).
Per core, per (img, group, och-pair-of-2x128, sptile-of-8-rows):
  - 8 full fp16 matmuls (K=128: two conv taps x 64 ic stacked; x stored as [A | A+1]
    and [A | A+58] shifted copies so taps pair up) + 2 row-tiled K=64 matmuls (tap
    (2,2) for the even/odd och tile packed into PE row groups 0-63/64-127, issued
    back-to-back so they overlap in the array) accumulate the 3x3 conv into two
    PSUM tiles [128 och, 448=8*56 cols] (pad columns skipped via 2D rhs AP).
  - ACT: Copy(psum * (w_scale/ps_scale)) with int8 output = round-half-even +
    saturate to [-128,127] in one op (verified on HW) == the LSQ psum quantizer.
  - DVE scalar_tensor_tensor: acc_fp16 += q_int8 * (+-ps_scale).
Output fp16 -> host f32. img0 runs group-outer (startup-friendly: only g0's x
needed first), img1 runs sp-outer/group-inner so the tail drains fast.
"""

import numpy as np

import concourse.mybir as mybir
import concourse.tile as tile
from concourse import bacc
from concourse.bass_utils import run_bass_kernel_spmd

F32 = mybir.dt.float32
F16 = mybir.dt.float16
I8 = mybir.dt.int8

B, IC, H, W = 16, 256, 56, 56
OC = 256
G = 4
CG = 64  # ic per group
K = 3
QP_W = 3  # 2**2 - 1
N_CORES = 8
BPC = B // N_CORES  # images per core

PW = W + 2  # 58 padded width
PH = H + 2
FLAT = PW * PH  # 3364
FLATP = FLAT + 4  # padded to 3368 for tap-read overhang
SP = 7  # spatial tiles of 8 output rows
ROWS = 8
NCOL = ROWS * W  # 448 real output columns per psum tile (pad cols skipped)
OC4 = 4  # och tiles of 128 over 512 (pos|neg x 256)
WT = 18  # weight tiles of 128 cols per group: 4x4 full slots + 2 paired slot4
WBASE = {0: 0, 1: 4, 2: 9, 3: 13}  # full-slot block base per oc4

_nc_cache = {}


def _build_nc():
    nc = bacc.Bacc(
        "TRN2",
        target_bir_lowering=False,
        debug=False,
        enable_asserts=True,
        num_devices=N_CORES,
    )

    xt1_d = nc.dram_tensor("xt1", [BPC, G, 128, FLATP], F16, kind="ExternalInput").ap()
    xt2_d = nc.dram_tensor("xt2", [BPC, G, 128, FLATP], F16, kind="ExternalInput").ap()
    wts_d = nc.dram_tensor("wts", [128, G * WT * 128], F16, kind="ExternalInput").ap()
    scl_d = nc.dram_tensor("scl", [128, 2 * G * OC4], F32, kind="ExternalInput").ap()
    out_d = nc.dram_tensor("out", [BPC, 2, SP, 128, NCOL], F16, kind="ExternalOutput").ap()

    with tile.TileContext(nc) as tc:
        with (
            tc.tile_pool(name="xp", bufs=1) as xp,
            tc.tile_pool(name="wp", bufs=1) as wp,
            tc.tile_pool(name="accp", bufs=2) as accp,
            tc.tile_pool(name="qp", bufs=8) as qp,
            tc.tile_pool(name="psum", bufs=8, space="PSUM") as pp,
        ):
            wts = wp.tile([128, G * WT * 128], F16, tag="wts")
            scl = wp.tile([128, 2 * G * OC4], F32, tag="scl")
            # Startup-critical DMA schedule over two queues (sync = HWDGE, gpsimd =
            # SWDGE). img0/g0 runs sp-outer pair-inner, so the first matmuls need
            # only w tiles 0-3 (oc4=0 slots) + the first ~640 x cols; order chunks
            # to release the first matmul as early as possible.
            WG = WT * 128  # weight cols per group
            C0, C1 = 640, 2000  # x-tile column chunk boundaries

            xt = {}
            t1_first = xp.tile([128, FLATP], F16, tag="t1_0_0")
            t2_first = xp.tile([128, FLATP], F16, tag="t2_0_0")
            xt[0, 0] = (t1_first, t2_first)

            nc.sync.dma_start(wts[:, : 4 * 128], wts_d[:, : 4 * 128])
            nc.sync.dma_start(t1_first[:, :C0], xt1_d[0, 0, :, :C0])
            nc.sync.dma_start(wts[:, 4 * 128 : 9 * 128], wts_d[:, 4 * 128 : 9 * 128])
            nc.sync.dma_start(scl[:], scl_d[:])
            nc.sync.dma_start(wts[:, 9 * 128 : WG], wts_d[:, 9 * 128 : WG])
            nc.sync.dma_start(t1_first[:, C0:C1], xt1_d[0, 0, :, C0:C1])
            nc.sync.dma_start(t1_first[:, C1:], xt1_d[0, 0, :, C1:])

            nc.gpsimd.dma_start(t2_first[:, :C0], xt2_d[0, 0, :, :C0])
            nc.gpsimd.dma_start(t2_first[:, C0:C1], xt2_d[0, 0, :, C0:C1])
            nc.gpsimd.dma_start(t2_first[:, C1:], xt2_d[0, 0, :, C1:])
            nc.gpsimd.dma_start(wts[:, WG : 2 * WG], wts_d[:, WG : 2 * WG])

            for img in range(BPC):
                for g in range(G):
                    if (img, g) in xt:
                        continue
                    t1 = xp.tile([128, FLATP], F16, tag=f"t1_{img}_{g}")
                    t2 = xp.tile([128, FLATP], F16, tag=f"t2_{img}_{g}")
                    nc.sync.dma_start(t1[:], xt1_d[img, g])
                    nc.gpsimd.dma_start(t2[:], xt2_d[img, g])
                    xt[img, g] = (t1, t2)
                    if (img, g) == (0, 1):
                        # remaining weights after the (0,1) x tiles
                        nc.gpsimd.dma_start(wts[:, 2 * WG :], wts_d[:, 2 * WG :])

            def wfull(g, oc4, s):
                i = g * WT + WBASE[oc4] + s
                return wts[:, i * 128 : (i + 1) * 128]

            def wpaircol(g, pair):
                i = g * WT + (8 if pair == 0 else 17)
                return i * 128

            def rhs2d(t, base):
                # [p, 8 rows stride PW, 56 cols] view skipping the 2 pad cols/row
                return t[:, base : base + ROWS * PW].rearrange(
                    "p (r c) -> p r c", c=PW
                )[:, :, :W]

            acc_of = {}

            def conv_block(img, g, pair, sp, acc):
                """2 och tiles (oc4 = pair*2 + {0,1}): 8 full MMs + 2 row-tiled
                K=64 MMs -> 2 psum tiles -> quantize (ACT) -> accumulate (DVE)."""
                oc4e, oc4o = (0, 1) if pair == 0 else (2, 3)
                t1, t2 = xt[img, g]
                r0 = sp * ROWS
                psE = pp.tile([128, NCOL], F32, tag="ps")
                psO = pp.tile([128, NCOL], F32, tag="ps")
                for oc4, ps in ((oc4e, psE), (oc4o, psO)):
                    for s in range(3):
                        nc.tensor.matmul(
                            ps[:],
                            wfull(g, oc4, s),
                            rhs2d(t1, (r0 + s) * PW),
                            start=(s == 0),
                            stop=False,
                        )
                    nc.tensor.matmul(
                        ps[:],
                        wfull(g, oc4, 3),
                        rhs2d(t2, r0 * PW + 2),
                        start=False,
                        stop=False,
                    )
                # tap (2,2) for both och tiles, packed into PE row groups
                # 0-63 / 64-127 (upper x half holds A+1, hence the -1 col offset)
                wc = wpaircol(g, pair)
                nc.tensor.matmul(
                    psE[:],
                    wts[0:64, wc : wc + 128],
                    rhs2d(t1[0:64], (r0 + 2) * PW + 2),
                    start=False,
                    stop=True,
                    tile_position=(0, 0),
                )
                nc.tensor.matmul(
                    psO[:],
                    wts[64:128, wc : wc + 128],
                    rhs2d(t1[64:128], (r0 + 2) * PW + 1),
                    start=False,
                    stop=True,
                    tile_position=(64, 0),
                )

                for oc4, ps in ((oc4e, psE), (oc4o, psO)):
                    iscl = g * OC4 + oc4
                    ratio_ap = scl[:, iscl : iscl + 1]
                    c_ap = scl[:, G * OC4 + iscl : G * OC4 + iscl + 1]
                    q8 = qp.tile([128, NCOL], I8, tag="q8")
                    nc.scalar.activation(
                        q8[:],
                        ps[:],
                        mybir.ActivationFunctionType.Copy,
                        bias=0.0,
                        scale=ratio_ap,
                    )
                    a = acc[oc4 % 2, sp]
                    if g == 0 and pair == 0:
                        nc.vector.tensor_scalar(
                            a[:], q8[:], c_ap, None, mybir.AluOpType.mult
                        )
                    else:
                        nc.vector.scalar_tensor_tensor(
                            a[:],
                            q8[:],
                            c_ap,
                            a[:],
                            mybir.AluOpType.mult,
                            mybir.AluOpType.add,
                        )

            def dma_out(img, sp, acc, n):
                for oct in range(2):
                    eng = nc.sync if (n + oct) % 2 == 0 else nc.gpsimd
                    eng.dma_start(out_d[img, oct, sp], acc[oct, sp][:])

            # img0: group-outer (x tiles stream in per group), sp-outer inside
            acc0 = {}
            for oct in range(2):
                for sp in range(SP):
                    acc0[oct, sp] = accp.tile([128, NCOL], F16, tag=f"acc{oct}_{sp}", name=f"acc0_{oct}_{sp}")
            for g in range(G):
                for sp in range(SP):
                    for pair in range(2):
                        conv_block(0, g, pair, sp, acc0)
                    if g == G - 1:
                        dma_out(0, sp, acc0, sp)

            # img1: sp-outer, group-inner -> each sp tile fully drains early
            acc1 = {}
            for oct in range(2):
                for sp in range(SP):
                    acc1[oct, sp] = accp.tile([128, NCOL], F16, tag=f"acc{oct}_{sp}", name=f"acc1_{oct}_{sp}")
            for sp in range(SP):
                for g in range(G):
                    for pair in range(2):
                        conv_block(1, g, pair, sp, acc1)
                dma_out(1, sp, acc1, sp)

    nc.compile()
    return nc


def _prepare(x, weight, w_scale, ps_scale_p, ps_scale_n):
    x = np.asarray(x, np.float32)
    weight = np.asarray(weight, np.float32)
    w_scale = np.asarray(w_scale, np.float32)
    ps_scale_p = np.asarray(ps_scale_p, np.float32)
    ps_scale_n = np.asarray(ps_scale_n, np.float32)

    # --- weight levels (exact f32 math matching the reference LSQ) ---
    wg = weight.reshape(OC, G, CG, K, K).transpose(1, 0, 2, 3, 4)  # [G,O,cg,k,k]
    s_w = w_scale.reshape(G, 1, 1, 1, 1)
    lvl_p = np.round(np.clip(np.maximum(wg, 0) / s_w, 0.0, float(QP_W))).astype(np.float32)
    lvl_n = np.round(np.clip(np.maximum(-wg, 0) / s_w, 0.0, float(QP_W))).astype(np.float32)
    LV = np.concatenate([lvl_p, lvl_n], axis=1)  # [G, 512, cg, 3, 3]

    # lhsT tiles [K, M=128]: per (g, oc4) 4 full K=128 slots (taps paired via the
    # shifted x copies) + per (g, och-pair) one slot4 tile holding tap (2,2) for
    # the even oc4 in partitions 0-63 and the odd oc4 in partitions 64-127.
    wts = np.zeros((G, WT, 128, 128), np.float16)
    for g in range(G):
        for oc4 in range(OC4):
            t = LV[g, oc4 * 128 : (oc4 + 1) * 128]  # [128 och, cg, 3, 3]
            b = WBASE[oc4]
            for s in range(3):  # taps (s,0)+(s,1)
                wts[g, b + s, :CG] = t[:, :, s, 0].T
                wts[g, b + s, CG:] = t[:, :, s, 1].T
            wts[g, b + 3, :CG] = t[:, :, 0, 2].T  # taps (0,2)+(1,2) via T2
            wts[g, b + 3, CG:] = t[:, :, 1, 2].T
            pi = 8 if oc4 < 2 else 17
            half = slice(0, CG) if oc4 % 2 == 0 else slice(CG, 128)
            wts[g, pi, half] = t[:, :, 2, 2].T  # tap (2,2), row-tiled pair
    # -> [128 K, G*WT*128]
    wts_flat = np.ascontiguousarray(wts.transpose(2, 0, 1, 3).reshape(128, G * WT * 128))

    # --- scales: ratio = s_w/s_ps ; c = +-s_ps ---
    scl = np.zeros((128, 2 * G * OC4), np.float32)
    for g in range(G):
        for oc4 in range(OC4):
            s_ps = ps_scale_p[g] if oc4 < 2 else ps_scale_n[g]
            sign = 1.0 if oc4 < 2 else -1.0
            scl[:, g * OC4 + oc4] = np.float32(w_scale[g]) / np.float32(s_ps)
            scl[:, G * OC4 + g * OC4 + oc4] = np.float32(sign) * np.float32(s_ps)

    # --- padded, shifted x in fp16 ---
    xp = np.zeros((B, IC, PH, PW), np.float16)
    xp[:, :, 1 : H + 1, 1 : W + 1] = x.astype(np.float16)
    Af = np.zeros((B, G, CG, FLATP), np.float16)
    Af[..., :FLAT] = xp.reshape(B, G, CG, FLAT)
    T1 = np.zeros((B, G, 128, FLATP), np.float16)
    T1[:, :, :CG] = Af
    T1[:, :, CG:, : FLATP - 1] = Af[..., 1:]
    T2 = np.zeros((B, G, 128, FLATP), np.float16)
    T2[:, :, :CG] = Af
    T2[:, :, CG:, : FLATP - PW] = Af[..., PW:]

    return T1, T2, wts_flat, scl


def kernel(x, weight, w_scale, ps_scale_p, ps_scale_n, _trace=False, _tmpdir=None):
    T1, T2, wts_flat, scl = _prepare(x, weight, w_scale, ps_scale_p, ps_scale_n)

    if "nc" not in _nc_cache:
        _nc_cache["nc"] = _build_nc()
    nc = _nc_cache["nc"]

    in_maps = []
    for c in range(N_CORES):
        sl = slice(c * BPC, (c + 1) * BPC)
        in_maps.append(
            {
                "xt1": np.ascontiguousarray(T1[sl]),
                "xt2": np.ascontiguousarray(T2[sl]),
                "wts": wts_flat,
                "scl": scl,
            }
        )

    kwargs = {}
    if _trace:
        kwargs.update(trace=True, tmpdir=_tmpdir, trace_cores=[0])
    res = run_bass_kernel_spmd(nc, in_maps, core_ids=list(range(N_CORES)), **kwargs)

    out = np.concatenate([r["out"] for r in res.results], axis=0)  # [16, 2, 7, 128, 448] fp16
    v = out.reshape(B, 2, SP, 128, ROWS, W)
    final = np.ascontiguousarray(v.transpose(0, 1, 3, 2, 4, 5)).reshape(B, OC, H, W).astype(np.float32)
    if _trace:
        kernel._last_results = res
    return final


# revision 4
# speedup vs baseline: 1.0891x; 1.0730x over previous
"""Trainium2 Bass kernel for nn_Conv4Pim_group_arr_v3 (PIM-style grouped quantized conv).

Computation (see reference):
  - x [16,256,56,56] f32, weight [256,256,3,3], per-group (G=4, 64 ic each) LSQ
    quantization: weights to integer levels {0..3} (pos/neg split), partial-sum conv
    outputs rounded to int levels in [-128,127] and rescaled, accumulated over groups.

Strategy: data-parallel over batch (2 images per core, 8 cores, no collectives).
Per core, per (img, group, och-pair-of-2x128, sptile-of-8-rows):
  - 8 full fp16 matmuls (K=128: two conv taps x 64 ic stacked; x stored as [A | A+1]
    and [A | A+58] shifted copies so taps pair up) + 2 row-tiled K=64 matmuls (tap
    (2,2) for the even/odd och tile packed into PE row groups 0-63/64-127, issued
    back-to-back so they overlap in the array) accumulate the 3x3 conv into two
    PSUM tiles [128 och, 448=8*56 cols] (pad columns skipped via 2D rhs AP).
  - ACT: Copy(psum * (w_scale/ps_scale)) with int8 output = round-half-even +
    saturate to [-128,127] in one op (verified on HW) == the LSQ psum quantizer.
  - DVE scalar_tensor_tensor: acc_fp16 += q_int8 * (+-ps_scale).
Output fp16 -> host f32. img0 runs group-outer (startup-friendly: only g0's x
needed first), img1 runs sp-outer/group-inner so the tail drains fast.
"""

import numpy as np

import concourse.mybir as mybir
import concourse.tile as tile
from concourse import bacc
from concourse.bass_utils import run_bass_kernel_spmd

F32 = mybir.dt.float32
F16 = mybir.dt.float16
I8 = mybir.dt.int8

B, IC, H, W = 16, 256, 56, 56
OC = 256
G = 4
CG = 64  # ic per group
K = 3
QP_W = 3  # 2**2 - 1
N_CORES = 8
BPC = B // N_CORES  # images per core

PW = W + 2  # 58 padded width
PH = H + 2
FLAT = PW * PH  # 3364
FLATP = FLAT + 4  # padded to 3368 for tap-read overhang
SP = 7  # spatial tiles of 8 output rows
ROWS = 8
NCOL = ROWS * W  # 448 real output columns per psum tile (pad cols skipped)
OC4 = 4  # och tiles of 128 over 512 (pos|neg x 256)
WT = 18  # weight tiles of 128 cols per group: 4x4 full slots + 2 paired slot4
WBASE = {0: 0, 1: 4, 2: 9, 3: 13}  # full-slot block base per oc4

_nc_cache = {}


def _build_nc():
    nc = bacc.Bacc(
        "TRN2",
        target_bir_lowering=False,
        debug=False,
        enable_asserts=True,
        num_devices=N_CORES,
    )

    xt1_d = nc.dram_tensor("xt1", [BPC, G, 128, FLATP], F16, kind="ExternalInput").ap()
    xt2_d = nc.dram_tensor("xt2", [BPC, G, 128, FLATP], F16, kind="ExternalInput").ap()
    wts_d = nc.dram_tensor("wts", [128, G * WT * 128], F16, kind="ExternalInput").ap()
    scl_d = nc.dram_tensor("scl", [128, 2 * G * OC4], F32, kind="ExternalInput").ap()
    out_d = nc.dram_tensor("out", [BPC, 2, SP, 128, NCOL], F16, kind="ExternalOutput").ap()

    with tile.TileContext(nc) as tc:
        with (
            tc.tile_pool(name="xp", bufs=1) as xp,
            tc.tile_pool(name="wp", bufs=1) as wp,
            tc.tile_pool(name="accp", bufs=2) as accp,
            tc.tile_pool(name="qp", bufs=8) as qp,
            tc.tile_pool(name="psum", bufs=8, space="PSUM") as pp,
        ):
            wts = wp.tile([128, G * WT * 128], F16, tag="wts")
            scl = wp.tile([128, 2 * G * OC4], F32, tag="scl")
            # Startup-critical DMA schedule over two queues (sync = HWDGE, gpsimd =
            # SWDGE). img0/g0 runs sp-outer pair-inner, so the first matmuls need
            # only w tiles 0-3 (oc4=0 slots) + the first ~640 x cols; order chunks
            # to release the first matmul as early as possible.
            WG = WT * 128  # weight cols per group
            C0, C1 = 640, 2000  # x-tile column chunk boundaries

            xt = {}
            t1_first = xp.tile([128, FLATP], F16, tag="t1_0_0")
            t2_first = xp.tile([128, FLATP], F16, tag="t2_0_0")
            xt[0, 0] = (t1_first, t2_first)

            # critical path split across both queue families: sync carries t1,
            # gpsimd carries the first weight tiles + t2
            nc.sync.dma_start(t1_first[:, :C0], xt1_d[0, 0, :, :C0])
            nc.sync.dma_start(wts[:, 4 * 128 : 9 * 128], wts_d[:, 4 * 128 : 9 * 128])
            nc.sync.dma_start(wts[:, 9 * 128 : WG], wts_d[:, 9 * 128 : WG])
            nc.sync.dma_start(scl[:], scl_d[:])
            nc.sync.dma_start(t1_first[:, C0:C1], xt1_d[0, 0, :, C0:C1])
            nc.sync.dma_start(t1_first[:, C1:], xt1_d[0, 0, :, C1:])

            nc.gpsimd.dma_start(wts[:, : 4 * 128], wts_d[:, : 4 * 128])
            nc.gpsimd.dma_start(t2_first[:, :C0], xt2_d[0, 0, :, :C0])
            nc.gpsimd.dma_start(t2_first[:, C0:C1], xt2_d[0, 0, :, C0:C1])
            nc.gpsimd.dma_start(t2_first[:, C1:], xt2_d[0, 0, :, C1:])
            nc.gpsimd.dma_start(wts[:, WG : 2 * WG], wts_d[:, WG : 2 * WG])

            for img in range(BPC):
                for g in range(G):
                    if (img, g) in xt:
                        continue
                    t1 = xp.tile([128, FLATP], F16, tag=f"t1_{img}_{g}")
                    t2 = xp.tile([128, FLATP], F16, tag=f"t2_{img}_{g}")
                    nc.sync.dma_start(t1[:], xt1_d[img, g])
                    nc.gpsimd.dma_start(t2[:], xt2_d[img, g])
                    xt[img, g] = (t1, t2)
                    if (img, g) == (0, 1):
                        # remaining weights after the (0,1) x tiles
                        nc.gpsimd.dma_start(wts[:, 2 * WG :], wts_d[:, 2 * WG :])

            def wfull(g, oc4, s):
                i = g * WT + WBASE[oc4] + s
                return wts[:, i * 128 : (i + 1) * 128]

            def wpaircol(g, pair):
                i = g * WT + (8 if pair == 0 else 17)
                return i * 128

            def rhs2d(t, base):
                # [p, 8 rows stride PW, 56 cols] view skipping the 2 pad cols/row
                return t[:, base : base + ROWS * PW].rearrange(
                    "p (r c) -> p r c", c=PW
                )[:, :, :W]

            def super_block(img, g, sp, acc, init):
                """All 4 och tiles of one (img, g, sp): 16 full MMs, then the
                4 row-tiled K=64 tap-(2,2) MMs back-to-back (amortizes the
                LDWEIGHTS-exposure penalty of full<->row-tiled transitions),
                then per-tile quantize (ACT) + accumulate (DVE)."""
                t1, t2 = xt[img, g]
                r0 = sp * ROWS
                ps = {}
                for oc4 in range(OC4):
                    p = pp.tile([128, NCOL], F32, tag="ps", name=f"ps{oc4}")
                    ps[oc4] = p
                    for s_ in range(3):
                        nc.tensor.matmul(
                            p[:],
                            wfull(g, oc4, s_),
                            rhs2d(t1, (r0 + s_) * PW),
                            start=(s_ == 0),
                            stop=False,
                        )
                    nc.tensor.matmul(
                        p[:],
                        wfull(g, oc4, 3),
                        rhs2d(t2, r0 * PW + 2),
                        start=False,
                        stop=False,
                    )
                # tap (2,2) x4, row groups 0-63 / 64-127 alternating (upper x
                # half holds A+1, hence the -1 col offset for odd oc4)
                for pair in range(2):
                    wc = wpaircol(g, pair)
                    nc.tensor.matmul(
                        ps[2 * pair][:],
                        wts[0:64, wc : wc + 128],
                        rhs2d(t1[0:64], (r0 + 2) * PW + 2),
                        start=False,
                        stop=True,
                        tile_position=(0, 0),
                    )
                    nc.tensor.matmul(
                        ps[2 * pair + 1][:],
                        wts[64:128, wc : wc + 128],
                        rhs2d(t1[64:128], (r0 + 2) * PW + 1),
                        start=False,
                        stop=True,
                        tile_position=(64, 0),
                    )

                for oc4 in range(OC4):
                    iscl = g * OC4 + oc4
                    ratio_ap = scl[:, iscl : iscl + 1]
                    c_ap = scl[:, G * OC4 + iscl : G * OC4 + iscl + 1]
                    q8 = qp.tile([128, NCOL], I8, tag="q8")
                    nc.scalar.activation(
                        q8[:],
                        ps[oc4][:],
                        mybir.ActivationFunctionType.Copy,
                        bias=0.0,
                        scale=ratio_ap,
                    )
                    a = acc[oc4 % 2, sp]
                    if init and oc4 < 2:
                        nc.vector.tensor_scalar(
                            a[:], q8[:], c_ap, None, mybir.AluOpType.mult
                        )
                    else:
                        nc.vector.scalar_tensor_tensor(
                            a[:],
                            q8[:],
                            c_ap,
                            a[:],
                            mybir.AluOpType.mult,
                            mybir.AluOpType.add,
                        )

            def dma_out(img, sp, acc, n):
                for oct in range(2):
                    eng = nc.sync if (n + oct) % 2 == 0 else nc.gpsimd
                    eng.dma_start(out_d[img, oct, sp], acc[oct, sp][:])

            # img0: group-outer (x tiles stream in per group), sp-outer inside
            acc0 = {}
            for oct in range(2):
                for sp in range(SP):
                    acc0[oct, sp] = accp.tile([128, NCOL], F16, tag=f"acc{oct}_{sp}", name=f"acc0_{oct}_{sp}")
            for g in range(G):
                for sp in range(SP):
                    super_block(0, g, sp, acc0, init=(g == 0))
                    if g == G - 1:
                        dma_out(0, sp, acc0, sp)

            # img1: sp-outer, group-inner -> each sp tile fully drains early
            acc1 = {}
            for oct in range(2):
                for sp in range(SP):
                    acc1[oct, sp] = accp.tile([128, NCOL], F16, tag=f"acc{oct}_{sp}", name=f"acc1_{oct}_{sp}")
            for sp in range(SP):
                for g in range(G):
                    super_block(1, g, sp, acc1, init=(g == 0))
                dma_out(1, sp, acc1, sp)

    nc.compile()
    return nc


def _prepare(x, weight, w_scale, ps_scale_p, ps_scale_n):
    x = np.asarray(x, np.float32)
    weight = np.asarray(weight, np.float32)
    w_scale = np.asarray(w_scale, np.float32)
    ps_scale_p = np.asarray(ps_scale_p, np.float32)
    ps_scale_n = np.asarray(ps_scale_n, np.float32)

    # --- weight levels (exact f32 math matching the reference LSQ) ---
    wg = weight.reshape(OC, G, CG, K, K).transpose(1, 0, 2, 3, 4)  # [G,O,cg,k,k]
    s_w = w_scale.reshape(G, 1, 1, 1, 1)
    lvl_p = np.round(np.clip(np.maximum(wg, 0) / s_w, 0.0, float(QP_W))).astype(np.float32)
    lvl_n = np.round(np.clip(np.maximum(-wg, 0) / s_w, 0.0, float(QP_W))).astype(np.float32)
    LV = np.concatenate([lvl_p, lvl_n], axis=1)  # [G, 512, cg, 3, 3]

    # lhsT tiles [K, M=128]: per (g, oc4) 4 full K=128 slots (taps paired via the
    # shifted x copies) + per (g, och-pair) one slot4 tile holding tap (2,2) for
    # the even oc4 in partitions 0-63 and the odd oc4 in partitions 64-127.
    wts = np.zeros((G, WT, 128, 128), np.float16)
    for g in range(G):
        for oc4 in range(OC4):
            t = LV[g, oc4 * 128 : (oc4 + 1) * 128]  # [128 och, cg, 3, 3]
            b = WBASE[oc4]
            for s in range(3):  # taps (s,0)+(s,1)
                wts[g, b + s, :CG] = t[:, :, s, 0].T
                wts[g, b + s, CG:] = t[:, :, s, 1].T
            wts[g, b + 3, :CG] = t[:, :, 0, 2].T  # taps (0,2)+(1,2) via T2
            wts[g, b + 3, CG:] = t[:, :, 1, 2].T
            pi = 8 if oc4 < 2 else 17
            half = slice(0, CG) if oc4 % 2 == 0 else slice(CG, 128)
            wts[g, pi, half] = t[:, :, 2, 2].T  # tap (2,2), row-tiled pair
    # -> [128 K, G*WT*128]
    wts_flat = np.ascontiguousarray(wts.transpose(2, 0, 1, 3).reshape(128, G * WT * 128))

    # --- scales: ratio = s_w/s_ps ; c = +-s_ps ---
    scl = np.zeros((128, 2 * G * OC4), np.float32)
    for g in range(G):
        for oc4 in range(OC4):
            s_ps = ps_scale_p[g] if oc4 < 2 else ps_scale_n[g]
            sign = 1.0 if oc4 < 2 else -1.0
            scl[:, g * OC4 + oc4] = np.float32(w_scale[g]) / np.float32(s_ps)
            scl[:, G * OC4 + g * OC4 + oc4] = np.float32(sign) * np.float32(s_ps)

    # --- padded, shifted x in fp16 ---
    xp = np.zeros((B, IC, PH, PW), np.float16)
    xp[:, :, 1 : H + 1, 1 : W + 1] = x.astype(np.float16)
    Af = np.zeros((B, G, CG, FLATP), np.float16)
    Af[..., :FLAT] = xp.reshape(B, G, CG, FLAT)
    T1 = np.zeros((B, G, 128, FLATP), np.float16)
    T1[:, :, :CG] = Af
    T1[:, :, CG:, : FLATP - 1] = Af[..., 1:]
    T2 = np.zeros((B, G, 128, FLATP), np.float16)
    T2[:, :, :CG] = Af
    T2[:, :, CG:, : FLATP - PW] = Af[..., PW:]

    return T1, T2, wts_flat, scl


def kernel(x, weight, w_scale, ps_scale_p, ps_scale_n, _trace=False, _tmpdir=None):
    T1, T2, wts_flat, scl = _prepare(x, weight, w_scale, ps_scale_p, ps_scale_n)

    if "nc" not in _nc_cache:
        _nc_cache["nc"] = _build_nc()
    nc = _nc_cache["nc"]

    in_maps = []
    for c in range(N_CORES):
        sl = slice(c * BPC, (c + 1) * BPC)
        in_maps.append(
            {
                "xt1": np.ascontiguousarray(T1[sl]),
                "xt2": np.ascontiguousarray(T2[sl]),
                "wts": wts_flat,
                "scl": scl,
            }
        )

    kwargs = {}
    if _trace:
        kwargs.update(trace=True, tmpdir=_tmpdir, trace_cores=[0])
    res = run_bass_kernel_spmd(nc, in_maps, core_ids=list(range(N_CORES)), **kwargs)

    out = np.concatenate([r["out"] for r in res.results], axis=0)  # [16, 2, 7, 128, 448] fp16
    v = out.reshape(B, 2, SP, 128, ROWS, W)
    final = np.ascontiguousarray(v.transpose(0, 1, 3, 2, 4, 5)).reshape(B, OC, H, W).astype(np.float32)
    if _trace:
        kernel._last_results = res
    return final


# revision 6
# speedup vs baseline: 1.0905x; 1.0013x over previous
"""Trainium2 Bass kernel for nn_Conv4Pim_group_arr_v3 (PIM-style grouped quantized conv).

Computation (see reference):
  - x [16,256,56,56] f32, weight [256,256,3,3], per-group (G=4, 64 ic each) LSQ
    quantization: weights to integer levels {0..3} (pos/neg split), partial-sum conv
    outputs rounded to int levels in [-128,127] and rescaled, accumulated over groups.

Strategy: data-parallel over batch (2 images per core, 8 cores, no collectives).
Per core, per (img, group, och-pair-of-2x128, sptile-of-8-rows):
  - 8 full fp16 matmuls (K=128: two conv taps x 64 ic stacked; x stored as [A | A+1]
    and [A | A+58] shifted copies so taps pair up) + 2 row-tiled K=64 matmuls (tap
    (2,2) for the even/odd och tile packed into PE row groups 0-63/64-127, issued
    back-to-back so they overlap in the array) accumulate the 3x3 conv into two
    PSUM tiles [128 och, 448=8*56 cols] (pad columns skipped via 2D rhs AP).
  - ACT: Copy(psum * (w_scale/ps_scale)) with int8 output = round-half-even +
    saturate to [-128,127] in one op (verified on HW) == the LSQ psum quantizer.
  - DVE scalar_tensor_tensor: acc_fp16 += q_int8 * (+-ps_scale).
Output fp16 -> host f32. img0 runs group-outer (startup-friendly: only g0's x
needed first), img1 runs sp-outer/group-inner so the tail drains fast.
"""

import numpy as np

import concourse.mybir as mybir
import concourse.tile as tile
from concourse import bacc
from concourse.bass_utils import run_bass_kernel_spmd

F32 = mybir.dt.float32
F16 = mybir.dt.float16
I8 = mybir.dt.int8

B, IC, H, W = 16, 256, 56, 56
OC = 256
G = 4
CG = 64  # ic per group
K = 3
QP_W = 3  # 2**2 - 1
N_CORES = 8
BPC = B // N_CORES  # images per core

PW = W + 2  # 58 padded width
PH = H + 2
FLAT = PW * PH  # 3364
FLATP = FLAT + 4  # padded to 3368 for tap-read overhang
SP = 7  # spatial tiles of 8 output rows
ROWS = 8
NCOL = ROWS * W  # 448 real output columns per psum tile (pad cols skipped)
OC4 = 4  # och tiles of 128 over 512 (pos|neg x 256)
WT = 18  # weight tiles of 128 cols per group: 4x4 full slots + 2 paired slot4
WBASE = {0: 0, 1: 4, 2: 9, 3: 13}  # full-slot block base per oc4

_nc_cache = {}


def _build_nc():
    nc = bacc.Bacc(
        "TRN2",
        target_bir_lowering=False,
        debug=False,
        enable_asserts=True,
        num_devices=N_CORES,
    )

    xt1_d = nc.dram_tensor("xt1", [BPC, G, 128, FLATP], F16, kind="ExternalInput").ap()
    xt2_d = nc.dram_tensor("xt2", [BPC, G, 128, FLATP], F16, kind="ExternalInput").ap()
    wts_d = nc.dram_tensor("wts", [128, G * WT * 128], F16, kind="ExternalInput").ap()
    scl_d = nc.dram_tensor("scl", [128, 2 * G * OC4], F32, kind="ExternalInput").ap()
    out_d = nc.dram_tensor("out", [BPC, 2, SP, 128, NCOL], F16, kind="ExternalOutput").ap()

    with tile.TileContext(nc) as tc:
        with (
            tc.tile_pool(name="xp", bufs=1) as xp,
            tc.tile_pool(name="wp", bufs=1) as wp,
            tc.tile_pool(name="accp", bufs=2) as accp,
            tc.tile_pool(name="qp", bufs=8) as qp,
            tc.tile_pool(name="psum", bufs=8, space="PSUM") as pp,
        ):
            wts = wp.tile([128, G * WT * 128], F16, tag="wts")
            scl = wp.tile([128, 2 * G * OC4], F32, tag="scl")
            # Startup-critical DMA schedule over two queues (sync = HWDGE, gpsimd =
            # SWDGE). img0/g0 runs sp-outer pair-inner, so the first matmuls need
            # only w tiles 0-3 (oc4=0 slots) + the first ~640 x cols; order chunks
            # to release the first matmul as early as possible.
            WG = WT * 128  # weight cols per group
            C0, C1 = 640, 2000  # x-tile column chunk boundaries

            xt = {}
            t1_first = xp.tile([128, FLATP], F16, tag="t1_0_0")
            t2_first = xp.tile([128, FLATP], F16, tag="t2_0_0")
            xt[0, 0] = (t1_first, t2_first)

            # critical path split across both queue families: sync carries t1,
            # gpsimd carries the first weight tiles + t2
            # PE warm-up: dummy matmuls on a zeroed scratch tile keep the HAM
            # clock gate at full rate while the input DMAs land (PE would
            # otherwise start cold and re-throttle after >3.4us idle).
            wscr = wp.tile([128, 512], F16, tag="wscr")
            nc.vector.memset(wscr[:], 0.0)
            for _ in range(24):
                wps = pp.tile([128, NCOL], F32, tag="ps", name="wps")
                nc.tensor.matmul(wps[:], wscr[:, :128], wscr[:, :NCOL], start=True, stop=True)

            nc.sync.dma_start(t1_first[:, :C0], xt1_d[0, 0, :, :C0])
            nc.sync.dma_start(wts[:, 4 * 128 : 9 * 128], wts_d[:, 4 * 128 : 9 * 128])
            nc.sync.dma_start(t1_first[:, C0:C1], xt1_d[0, 0, :, C0:C1])
            nc.sync.dma_start(t1_first[:, C1:], xt1_d[0, 0, :, C1:])
            nc.scalar.dma_start(wts[:, 9 * 128 : WG], wts_d[:, 9 * 128 : WG])
            nc.scalar.dma_start(scl[:], scl_d[:])

            nc.gpsimd.dma_start(wts[:, : 4 * 128], wts_d[:, : 4 * 128])
            nc.gpsimd.dma_start(t2_first[:, :C0], xt2_d[0, 0, :, :C0])
            nc.gpsimd.dma_start(t2_first[:, C0:C1], xt2_d[0, 0, :, C0:C1])
            nc.gpsimd.dma_start(t2_first[:, C1:], xt2_d[0, 0, :, C1:])
            nc.gpsimd.dma_start(wts[:, WG : 2 * WG], wts_d[:, WG : 2 * WG])

            for img in range(BPC):
                for g in range(G):
                    if (img, g) in xt:
                        continue
                    t1 = xp.tile([128, FLATP], F16, tag=f"t1_{img}_{g}")
                    t2 = xp.tile([128, FLATP], F16, tag=f"t2_{img}_{g}")
                    nc.sync.dma_start(t1[:], xt1_d[img, g])
                    nc.gpsimd.dma_start(t2[:], xt2_d[img, g])
                    xt[img, g] = (t1, t2)
                    if (img, g) == (0, 1):
                        # remaining weights after the (0,1) x tiles
                        nc.gpsimd.dma_start(wts[:, 2 * WG :], wts_d[:, 2 * WG :])

            def wfull(g, oc4, s):
                i = g * WT + WBASE[oc4] + s
                return wts[:, i * 128 : (i + 1) * 128]

            def wpaircol(g, pair):
                i = g * WT + (8 if pair == 0 else 17)
                return i * 128

            def rhs2d(t, base):
                # [p, 8 rows stride PW, 56 cols] view skipping the 2 pad cols/row
                return t[:, base : base + ROWS * PW].rearrange(
                    "p (r c) -> p r c", c=PW
                )[:, :, :W]

            def super_block(img, g, sp, acc, init):
                """All 4 och tiles of one (img, g, sp): 16 full MMs, then the
                4 row-tiled K=64 tap-(2,2) MMs back-to-back (amortizes the
                LDWEIGHTS-exposure penalty of full<->row-tiled transitions),
                then per-tile quantize (ACT) + accumulate (DVE)."""
                t1, t2 = xt[img, g]
                r0 = sp * ROWS
                ps = {}
                for oc4 in range(OC4):
                    p = pp.tile([128, NCOL], F32, tag="ps", name=f"ps{oc4}")
                    ps[oc4] = p
                    for s_ in range(3):
                        nc.tensor.matmul(
                            p[:],
                            wfull(g, oc4, s_),
                            rhs2d(t1, (r0 + s_) * PW),
                            start=(s_ == 0),
                            stop=False,
                        )
                    nc.tensor.matmul(
                        p[:],
                        wfull(g, oc4, 3),
                        rhs2d(t2, r0 * PW + 2),
                        start=False,
                        stop=False,
                    )
                # tap (2,2) x4, row groups 0-63 / 64-127 alternating (upper x
                # half holds A+1, hence the -1 col offset for odd oc4)
                for pair in range(2):
                    wc = wpaircol(g, pair)
                    nc.tensor.matmul(
                        ps[2 * pair][:],
                        wts[0:64, wc : wc + 128],
                        rhs2d(t1[0:64], (r0 + 2) * PW + 2),
                        start=False,
                        stop=True,
                        tile_position=(0, 0),
                    )
                    nc.tensor.matmul(
                        ps[2 * pair + 1][:],
                        wts[64:128, wc : wc + 128],
                        rhs2d(t1[64:128], (r0 + 2) * PW + 1),
                        start=False,
                        stop=True,
                        tile_position=(64, 0),
                    )

                for oc4 in range(OC4):
                    iscl = g * OC4 + oc4
                    ratio_ap = scl[:, iscl : iscl + 1]
                    c_ap = scl[:, G * OC4 + iscl : G * OC4 + iscl + 1]
                    q8 = qp.tile([128, NCOL], I8, tag="q8")
                    nc.scalar.activation(
                        q8[:],
                        ps[oc4][:],
                        mybir.ActivationFunctionType.Copy,
                        bias=0.0,
                        scale=ratio_ap,
                    )
                    a = acc[oc4 % 2, sp]
                    if init and oc4 < 2:
                        nc.vector.tensor_scalar(
                            a[:], q8[:], c_ap, None, mybir.AluOpType.mult
                        )
                    else:
                        nc.vector.scalar_tensor_tensor(
                            a[:],
                            q8[:],
                            c_ap,
                            a[:],
                            mybir.AluOpType.mult,
                            mybir.AluOpType.add,
                        )

            def dma_out(img, sp, acc, n):
                for oct in range(2):
                    eng = nc.sync if (n + oct) % 2 == 0 else nc.gpsimd
                    eng.dma_start(out_d[img, oct, sp], acc[oct, sp][:])

            # img0: group-outer (x tiles stream in per group), sp-outer inside
            acc0 = {}
            for oct in range(2):
                for sp in range(SP):
                    acc0[oct, sp] = accp.tile([128, NCOL], F16, tag=f"acc{oct}_{sp}", name=f"acc0_{oct}_{sp}")
            for g in range(G):
                for sp in range(SP):
                    super_block(0, g, sp, acc0, init=(g == 0))
                    if g == G - 1:
                        dma_out(0, sp, acc0, sp)

            # img1: sp-outer, group-inner -> each sp tile fully drains early
            acc1 = {}
            for oct in range(2):
                for sp in range(SP):
                    acc1[oct, sp] = accp.tile([128, NCOL], F16, tag=f"acc{oct}_{sp}", name=f"acc1_{oct}_{sp}")
            for sp in range(SP):
                for g in range(G):
                    super_block(1, g, sp, acc1, init=(g == 0))
                dma_out(1, sp, acc1, sp)

    nc.compile()
    return nc


def _prepare(x, weight, w_scale, ps_scale_p, ps_scale_n):
    x = np.asarray(x, np.float32)
    weight = np.asarray(weight, np.float32)
    w_scale = np.asarray(w_scale, np.float32)
    ps_scale_p = np.asarray(ps_scale_p, np.float32)
    ps_scale_n = np.asarray(ps_scale_n, np.float32)

    # --- weight levels (exact f32 math matching the reference LSQ) ---
    wg = weight.reshape(OC, G, CG, K, K).transpose(1, 0, 2, 3, 4)  # [G,O,cg,k,k]
    s_w = w_scale.reshape(G, 1, 1, 1, 1)
    lvl_p = np.round(np.clip(np.maximum(wg, 0) / s_w, 0.0, float(QP_W))).astype(np.float32)
    lvl_n = np.round(np.clip(np.maximum(-wg, 0) / s_w, 0.0, float(QP_W))).astype(np.float32)
    LV = np.concatenate([lvl_p, lvl_n], axis=1)  # [G, 512, cg, 3, 3]

    # lhsT tiles [K, M=128]: per (g, oc4) 4 full K=128 slots (taps paired via the
    # shifted x copies) + per (g, och-pair) one slot4 tile holding tap (2,2) for
    # the even oc4 in partitions 0-63 and the odd oc4 in partitions 64-127.
    wts = np.zeros((G, WT, 128, 128), np.float16)
    for g in range(G):
        for oc4 in range(OC4):
            t = LV[g, oc4 * 128 : (oc4 + 1) * 128]  # [128 och, cg, 3, 3]
            b = WBASE[oc4]
            for s in range(3):  # taps (s,0)+(s,1)
                wts[g, b + s, :CG] = t[:, :, s, 0].T
                wts[g, b + s, CG:] = t[:, :, s, 1].T
            wts[g, b + 3, :CG] = t[:, :, 0, 2].T  # taps (0,2)+(1,2) via T2
            wts[g, b + 3, CG:] = t[:, :, 1, 2].T
            pi = 8 if oc4 < 2 else 17
            half = slice(0, CG) if oc4 % 2 == 0 else slice(CG, 128)
            wts[g, pi, half] = t[:, :, 2, 2].T  # tap (2,2), row-tiled pair
    # -> [128 K, G*WT*128]
    wts_flat = np.ascontiguousarray(wts.transpose(2, 0, 1, 3).reshape(128, G * WT * 128))

    # --- scales: ratio = s_w/s_ps ; c = +-s_ps ---
    scl = np.zeros((128, 2 * G * OC4), np.float32)
    for g in range(G):
        for oc4 in range(OC4):
            s_ps = ps_scale_p[g] if oc4 < 2 else ps_scale_n[g]
            sign = 1.0 if oc4 < 2 else -1.0
            scl[:, g * OC4 + oc4] = np.float32(w_scale[g]) / np.float32(s_ps)
            scl[:, G * OC4 + g * OC4 + oc4] = np.float32(sign) * np.float32(s_ps)

    # --- padded, shifted x in fp16 ---
    xp = np.zeros((B, IC, PH, PW), np.float16)
    xp[:, :, 1 : H + 1, 1 : W + 1] = x.astype(np.float16)
    Af = np.zeros((B, G, CG, FLATP), np.float16)
    Af[..., :FLAT] = xp.reshape(B, G, CG, FLAT)
    T1 = np.zeros((B, G, 128, FLATP), np.float16)
    T1[:, :, :CG] = Af
    T1[:, :, CG:, : FLATP - 1] = Af[..., 1:]
    T2 = np.zeros((B, G, 128, FLATP), np.float16)
    T2[:, :, :CG] = Af
    T2[:, :, CG:, : FLATP - PW] = Af[..., PW:]

    return T1, T2, wts_flat, scl


def kernel(x, weight, w_scale, ps_scale_p, ps_scale_n, _trace=False, _tmpdir=None):
    T1, T2, wts_flat, scl = _prepare(x, weight, w_scale, ps_scale_p, ps_scale_n)

    if "nc" not in _nc_cache:
        _nc_cache["nc"] = _build_nc()
    nc = _nc_cache["nc"]

    in_maps = []
    for c in range(N_CORES):
        sl = slice(c * BPC, (c + 1) * BPC)
        in_maps.append(
            {
                "xt1": np.ascontiguousarray(T1[sl]),
                "xt2": np.ascontiguousarray(T2[sl]),
                "wts": wts_flat,
                "scl": scl,
            }
        )

    kwargs = {}
    if _trace:
        kwargs.update(trace=True, tmpdir=_tmpdir, trace_cores=[0])
    res = run_bass_kernel_spmd(nc, in_maps, core_ids=list(range(N_CORES)), **kwargs)

    out = np.concatenate([r["out"] for r in res.results], axis=0)  # [16, 2, 7, 128, 448] fp16
    v = out.reshape(B, 2, SP, 128, ROWS, W)
    final = np.ascontiguousarray(v.transpose(0, 1, 3, 2, 4, 5)).reshape(B, OC, H, W).astype(np.float32)
    if _trace:
        kernel._last_results = res
    return final


# revision 8
# speedup vs baseline: 1.0932x; 1.0024x over previous
"""Trainium2 Bass kernel for nn_Conv4Pim_group_arr_v3 (PIM-style grouped quantized conv).

Computation (see reference):
  - x [16,256,56,56] f32, weight [256,256,3,3], per-group (G=4, 64 ic each) LSQ
    quantization: weights to integer levels {0..3} (pos/neg split), partial-sum conv
    outputs rounded to int levels in [-128,127] and rescaled, accumulated over groups.

Strategy: data-parallel over batch (2 images per core, 8 cores, no collectives).
Per core, per (img, group, och-pair-of-2x128, sptile-of-8-rows):
  - 8 full fp16 matmuls (K=128: two conv taps x 64 ic stacked; x stored as [A | A+1]
    and [A | A+58] shifted copies so taps pair up) + 2 row-tiled K=64 matmuls (tap
    (2,2) for the even/odd och tile packed into PE row groups 0-63/64-127, issued
    back-to-back so they overlap in the array) accumulate the 3x3 conv into two
    PSUM tiles [128 och, 448=8*56 cols] (pad columns skipped via 2D rhs AP).
  - ACT: Copy(psum * (w_scale/ps_scale)) with int8 output = round-half-even +
    saturate to [-128,127] in one op (verified on HW) == the LSQ psum quantizer.
  - DVE scalar_tensor_tensor: acc_fp16 += q_int8 * (+-ps_scale).
Output fp16 -> host f32. img0 runs group-outer (startup-friendly: only g0's x
needed first), img1 runs sp-outer/group-inner so the tail drains fast.
"""

import numpy as np

import concourse.mybir as mybir
import concourse.tile as tile
from concourse import bacc
from concourse.bass_utils import run_bass_kernel_spmd

F32 = mybir.dt.float32
F16 = mybir.dt.float16
I8 = mybir.dt.int8

B, IC, H, W = 16, 256, 56, 56
OC = 256
G = 4
CG = 64  # ic per group
K = 3
QP_W = 3  # 2**2 - 1
N_CORES = 8
BPC = B // N_CORES  # images per core

PW = W + 2  # 58 padded width
PH = H + 2
FLAT = PW * PH  # 3364
FLATP = FLAT + 4  # padded to 3368 for tap-read overhang
SP = 7  # spatial tiles of 8 output rows
ROWS = 8
NCOL = ROWS * W  # 448 real output columns per psum tile (pad cols skipped)
OC4 = 4  # och tiles of 128 over 512 (pos|neg x 256)
WT = 18  # weight tiles of 128 cols per group: 4x4 full slots + 2 paired slot4
WBASE = {0: 0, 1: 4, 2: 9, 3: 13}  # full-slot block base per oc4

_nc_cache = {}


def _build_nc():
    nc = bacc.Bacc(
        "TRN2",
        target_bir_lowering=False,
        debug=False,
        enable_asserts=True,
        num_devices=N_CORES,
    )

    xt1_d = nc.dram_tensor("xt1", [BPC, G, 128, FLATP], F16, kind="ExternalInput").ap()
    xt2_d = nc.dram_tensor("xt2", [BPC, G, 128, FLATP], F16, kind="ExternalInput").ap()
    wts_d = nc.dram_tensor("wts", [128, G * WT * 128], F16, kind="ExternalInput").ap()
    scl_d = nc.dram_tensor("scl", [128, 2 * G * OC4], F32, kind="ExternalInput").ap()
    out_d = nc.dram_tensor("out", [BPC, 2, SP, 128, NCOL], F16, kind="ExternalOutput").ap()

    with tile.TileContext(nc) as tc:
        with (
            tc.tile_pool(name="xp", bufs=1) as xp,
            tc.tile_pool(name="wp", bufs=1) as wp,
            tc.tile_pool(name="accp", bufs=2) as accp,
            tc.tile_pool(name="qp", bufs=8) as qp,
            tc.tile_pool(name="psum", bufs=8, space="PSUM") as pp,
        ):
            wts = wp.tile([128, G * WT * 128], F16, tag="wts")
            scl = wp.tile([128, 2 * G * OC4], F32, tag="scl")
            # Startup-critical DMA schedule over two queues (sync = HWDGE, gpsimd =
            # SWDGE). img0/g0 runs sp-outer pair-inner, so the first matmuls need
            # only w tiles 0-3 (oc4=0 slots) + the first ~640 x cols; order chunks
            # to release the first matmul as early as possible.
            WG = WT * 128  # weight cols per group
            C0, C1 = 640, 2000  # x-tile column chunk boundaries

            xt = {}
            t1_first = xp.tile([128, FLATP], F16, tag="t1_0_0")
            t2_first = xp.tile([128, FLATP], F16, tag="t2_0_0")
            xt[0, 0] = (t1_first, t2_first)

            # critical path split across both queue families: sync carries t1,
            # gpsimd carries the first weight tiles + t2
            # PE warm-up: dummy matmuls on a zeroed scratch tile keep the HAM
            # clock gate at full rate while the input DMAs land (PE would
            # otherwise start cold and re-throttle after >3.4us idle).
            wscr = wp.tile([128, 128], F16, tag="wscr")
            nc.vector.memset(wscr[:], 0.0)
            for _ in range(30):
                wps = pp.tile([128, NCOL], F32, tag="ps", name="wps")
                nc.tensor.matmul(wps[:, :64], wscr[:, :128], wscr[:, :64], start=True, stop=True)

            nc.sync.dma_start(t1_first[:, :C0], xt1_d[0, 0, :, :C0])
            nc.sync.dma_start(wts[:, 4 * 128 : 9 * 128], wts_d[:, 4 * 128 : 9 * 128])
            nc.sync.dma_start(t1_first[:, C0:C1], xt1_d[0, 0, :, C0:C1])
            nc.sync.dma_start(t1_first[:, C1:], xt1_d[0, 0, :, C1:])
            nc.scalar.dma_start(wts[:, 9 * 128 : WG], wts_d[:, 9 * 128 : WG])
            nc.scalar.dma_start(scl[:], scl_d[:])

            nc.gpsimd.dma_start(wts[:, : 4 * 128], wts_d[:, : 4 * 128])
            nc.gpsimd.dma_start(t2_first[:, :C0], xt2_d[0, 0, :, :C0])
            nc.gpsimd.dma_start(t2_first[:, C0:C1], xt2_d[0, 0, :, C0:C1])
            nc.gpsimd.dma_start(t2_first[:, C1:], xt2_d[0, 0, :, C1:])
            nc.gpsimd.dma_start(wts[:, WG : 2 * WG], wts_d[:, WG : 2 * WG])

            for img in range(BPC):
                for g in range(G):
                    if (img, g) in xt:
                        continue
                    t1 = xp.tile([128, FLATP], F16, tag=f"t1_{img}_{g}")
                    t2 = xp.tile([128, FLATP], F16, tag=f"t2_{img}_{g}")
                    nc.sync.dma_start(t1[:], xt1_d[img, g])
                    nc.gpsimd.dma_start(t2[:], xt2_d[img, g])
                    xt[img, g] = (t1, t2)
                    if (img, g) == (0, 1):
                        # remaining weights after the (0,1) x tiles
                        nc.gpsimd.dma_start(wts[:, 2 * WG :], wts_d[:, 2 * WG :])

            def wfull(g, oc4, s):
                i = g * WT + WBASE[oc4] + s
                return wts[:, i * 128 : (i + 1) * 128]

            def wpaircol(g, pair):
                i = g * WT + (8 if pair == 0 else 17)
                return i * 128

            def rhs2d(t, base):
                # [p, 8 rows stride PW, 56 cols] view skipping the 2 pad cols/row
                return t[:, base : base + ROWS * PW].rearrange(
                    "p (r c) -> p r c", c=PW
                )[:, :, :W]

            def quantize_acc(g, oc4, ps, acc, sp, init):
                iscl = g * OC4 + oc4
                ratio_ap = scl[:, iscl : iscl + 1]
                c_ap = scl[:, G * OC4 + iscl : G * OC4 + iscl + 1]
                q8 = qp.tile([128, NCOL], I8, tag="q8")
                nc.scalar.activation(
                    q8[:],
                    ps[oc4][:],
                    mybir.ActivationFunctionType.Copy,
                    bias=0.0,
                    scale=ratio_ap,
                )
                a = acc[oc4 % 2, sp]
                if init and oc4 < 2:
                    nc.vector.tensor_scalar(
                        a[:], q8[:], c_ap, None, mybir.AluOpType.mult
                    )
                else:
                    nc.vector.scalar_tensor_tensor(
                        a[:],
                        q8[:],
                        c_ap,
                        a[:],
                        mybir.AluOpType.mult,
                        mybir.AluOpType.add,
                    )

            def super_block(img, g, sp, acc, init, split=False):
                """All 4 och tiles of one (img, g, sp): 16 full MMs, then the
                4 row-tiled K=64 tap-(2,2) MMs back-to-back (amortizes the
                LDWEIGHTS-exposure penalty of full<->row-tiled transitions),
                then per-tile quantize (ACT) + accumulate (DVE)."""
                t1, t2 = xt[img, g]
                r0 = sp * ROWS
                ps = {}

                def mm_full(oc4):
                    p = pp.tile([128, NCOL], F32, tag="ps", name=f"ps{oc4}")
                    ps[oc4] = p
                    for s_ in range(3):
                        nc.tensor.matmul(
                            p[:],
                            wfull(g, oc4, s_),
                            rhs2d(t1, (r0 + s_) * PW),
                            start=(s_ == 0),
                            stop=False,
                        )
                    nc.tensor.matmul(
                        p[:],
                        wfull(g, oc4, 3),
                        rhs2d(t2, r0 * PW + 2),
                        start=False,
                        stop=False,
                    )

                def mm_tap22(pair):
                    # tap (2,2) for an och pair, row groups 0-63 / 64-127
                    # (upper x half holds A+1, hence the -1 col offset)
                    wc = wpaircol(g, pair)
                    nc.tensor.matmul(
                        ps[2 * pair][:],
                        wts[0:64, wc : wc + 128],
                        rhs2d(t1[0:64], (r0 + 2) * PW + 2),
                        start=False,
                        stop=True,
                        tile_position=(0, 0),
                    )
                    nc.tensor.matmul(
                        ps[2 * pair + 1][:],
                        wts[64:128, wc : wc + 128],
                        rhs2d(t1[64:128], (r0 + 2) * PW + 1),
                        start=False,
                        stop=True,
                        tile_position=(64, 0),
                    )

                if split:
                    # last superblock: finish pair0's psums early so their
                    # quantize runs under pair1's matmuls (shorter drain)
                    order = [(0, [0, 1]), (1, [2, 3])]
                else:
                    order = [(None, [0, 1, 2, 3])]
                for pair_group, oc4s in order:
                    for oc4 in oc4s:
                        mm_full(oc4)
                    if pair_group is None:
                        mm_tap22(0)
                        mm_tap22(1)
                    else:
                        mm_tap22(pair_group)
                    for oc4 in (oc4s if split else []):
                        quantize_acc(g, oc4, ps, acc, sp, init)
                if split:
                    return

                for oc4 in range(OC4):
                    quantize_acc(g, oc4, ps, acc, sp, init)

            def dma_out(img, sp, acc, n, last=False):
                if last:
                    HC = NCOL // 2
                    nc.sync.dma_start(out_d[img, 0, sp, :, :HC], acc[0, sp][:, :HC])
                    nc.gpsimd.dma_start(out_d[img, 0, sp, :, HC:], acc[0, sp][:, HC:])
                    nc.scalar.dma_start(out_d[img, 1, sp, :, :HC], acc[1, sp][:, :HC])
                    nc.sync.dma_start(out_d[img, 1, sp, :, HC:], acc[1, sp][:, HC:])
                    return
                for oct in range(2):
                    eng = nc.sync if (n + oct) % 2 == 0 else nc.gpsimd
                    eng.dma_start(out_d[img, oct, sp], acc[oct, sp][:])

            # img0: group-outer (x tiles stream in per group), sp-outer inside
            acc0 = {}
            for oct in range(2):
                for sp in range(SP):
                    acc0[oct, sp] = accp.tile([128, NCOL], F16, tag=f"acc{oct}_{sp}", name=f"acc0_{oct}_{sp}")
            for g in range(G):
                for sp in range(SP):
                    super_block(0, g, sp, acc0, init=(g == 0))
                    if g == G - 1:
                        dma_out(0, sp, acc0, sp)

            # img1: sp-outer, group-inner -> each sp tile fully drains early
            acc1 = {}
            for oct in range(2):
                for sp in range(SP):
                    acc1[oct, sp] = accp.tile([128, NCOL], F16, tag=f"acc{oct}_{sp}", name=f"acc1_{oct}_{sp}")
            for sp in range(SP):
                for g in range(G):
                    split = sp == SP - 1 and g == G - 1
                    super_block(1, g, sp, acc1, init=(g == 0), split=split)
                dma_out(1, sp, acc1, sp, last=(sp == SP - 1))

    nc.compile()
    return nc


def _prepare(x, weight, w_scale, ps_scale_p, ps_scale_n):
    x = np.asarray(x, np.float32)
    weight = np.asarray(weight, np.float32)
    w_scale = np.asarray(w_scale, np.float32)
    ps_scale_p = np.asarray(ps_scale_p, np.float32)
    ps_scale_n = np.asarray(ps_scale_n, np.float32)

    # --- weight levels (exact f32 math matching the reference LSQ) ---
    wg = weight.reshape(OC, G, CG, K, K).transpose(1, 0, 2, 3, 4)  # [G,O,cg,k,k]
    s_w = w_scale.reshape(G, 1, 1, 1, 1)
    lvl_p = np.round(np.clip(np.maximum(wg, 0) / s_w, 0.0, float(QP_W))).astype(np.float32)
    lvl_n = np.round(np.clip(np.maximum(-wg, 0) / s_w, 0.0, float(QP_W))).astype(np.float32)
    LV = np.concatenate([lvl_p, lvl_n], axis=1)  # [G, 512, cg, 3, 3]

    # lhsT tiles [K, M=128]: per (g, oc4) 4 full K=128 slots (taps paired via the
    # shifted x copies) + per (g, och-pair) one slot4 tile holding tap (2,2) for
    # the even oc4 in partitions 0-63 and the odd oc4 in partitions 64-127.
    wts = np.zeros((G, WT, 128, 128), np.float16)
    for g in range(G):
        for oc4 in range(OC4):
            t = LV[g, oc4 * 128 : (oc4 + 1) * 128]  # [128 och, cg, 3, 3]
            b = WBASE[oc4]
            for s in range(3):  # taps (s,0)+(s,1)
                wts[g, b + s, :CG] = t[:, :, s, 0].T
                wts[g, b + s, CG:] = t[:, :, s, 1].T
            wts[g, b + 3, :CG] = t[:, :, 0, 2].T  # taps (0,2)+(1,2) via T2
            wts[g, b + 3, CG:] = t[:, :, 1, 2].T
            pi = 8 if oc4 < 2 else 17
            half = slice(0, CG) if oc4 % 2 == 0 else slice(CG, 128)
            wts[g, pi, half] = t[:, :, 2, 2].T  # tap (2,2), row-tiled pair
    # -> [128 K, G*WT*128]
    wts_flat = np.ascontiguousarray(wts.transpose(2, 0, 1, 3).reshape(128, G * WT * 128))

    # --- scales: ratio = s_w/s_ps ; c = +-s_ps ---
    scl = np.zeros((128, 2 * G * OC4), np.float32)
    for g in range(G):
        for oc4 in range(OC4):
            s_ps = ps_scale_p[g] if oc4 < 2 else ps_scale_n[g]
            sign = 1.0 if oc4 < 2 else -1.0
            scl[:, g * OC4 + oc4] = np.float32(w_scale[g]) / np.float32(s_ps)
            scl[:, G * OC4 + g * OC4 + oc4] = np.float32(sign) * np.float32(s_ps)

    # --- padded, shifted x in fp16 ---
    xp = np.zeros((B, IC, PH, PW), np.float16)
    xp[:, :, 1 : H + 1, 1 : W + 1] = x.astype(np.float16)
    Af = np.zeros((B, G, CG, FLATP), np.float16)
    Af[..., :FLAT] = xp.reshape(B, G, CG, FLAT)
    T1 = np.zeros((B, G, 128, FLATP), np.float16)
    T1[:, :, :CG] = Af
    T1[:, :, CG:, : FLATP - 1] = Af[..., 1:]
    T2 = np.zeros((B, G, 128, FLATP), np.float16)
    T2[:, :, :CG] = Af
    T2[:, :, CG:, : FLATP - PW] = Af[..., PW:]

    return T1, T2, wts_flat, scl


def kernel(x, weight, w_scale, ps_scale_p, ps_scale_n, _trace=False, _tmpdir=None):
    T1, T2, wts_flat, scl = _prepare(x, weight, w_scale, ps_scale_p, ps_scale_n)

    if "nc" not in _nc_cache:
        _nc_cache["nc"] = _build_nc()
    nc = _nc_cache["nc"]

    in_maps = []
    for c in range(N_CORES):
        sl = slice(c * BPC, (c + 1) * BPC)
        in_maps.append(
            {
                "xt1": np.ascontiguousarray(T1[sl]),
                "xt2": np.ascontiguousarray(T2[sl]),
                "wts": wts_flat,
                "scl": scl,
            }
        )

    kwargs = {}
    if _trace:
        kwargs.update(trace=True, tmpdir=_tmpdir, trace_cores=[0])
    res = run_bass_kernel_spmd(nc, in_maps, core_ids=list(range(N_CORES)), **kwargs)

    out = np.concatenate([r["out"] for r in res.results], axis=0)  # [16, 2, 7, 128, 448] fp16
    v = out.reshape(B, 2, SP, 128, ROWS, W)
    final = np.ascontiguousarray(v.transpose(0, 1, 3, 2, 4, 5)).reshape(B, OC, H, W).astype(np.float32)
    if _trace:
        kernel._last_results = res
    return final
